# revision 29
# baseline (speedup 1.0000x reference)
"""Trainium2 Bass kernel for nn_AtariAgent57LitePolicy.

Data-parallel over 8 NeuronCores (batch 1024 -> 128 per core), no collectives.

Design (per-core):
  * All convs are dense TensorE matmuls with *image rows on partitions* and
    (out-col j, batch) on the streamed free dim.  The sparse Toeplitz band
    structure of each conv is expanded on the HOST into the stationary lhsT
    operands (weights are tiny), so the device only streams activations.
  * Everything TensorE-facing is bf16 (PSUM accumulation stays f32); x is
    pre-transposed to [h, c, chunk-blocked (w, b)] and cast to bf16 on the
    host, halving the HBM floor (~7 MB/core).
  * conv1: 3 input-row bands (32/32/20 rows x 4 ci -> K=128/128/80) x 8
    kernel-columns dx -> 24 accumulating matmuls into PSUM[80=(i,co), j, b],
    per batch-chunk (sizes [10,18,25,25,25,25]; N=20*bt <= 512 per matmul).
    Band-boundary output rows receive partial sums from two bands via PSUM.
  * conv2 (k4 s2, K=80, M=72) and conv3 (k3 s1, K=72, M=56) run once over
    the full 128 batch at the end (12 + 6 matmuls), then proj (7 matmuls)
    -> conv_feat [32, 128].
  * LSTM: h/c arrive host-transposed (feature-major); W_hh@h issues during
    chunk 0; prev_action one-hot, prev_reward, and both LSTM biases fold
    into one gate-bias vector on the host.  Gates are host-permuted to
    (i,f,o,g) and evaluated with a single Sigmoid pass (tanh(g) recovered
    as 2*sigmoid(2g)-1 via a per-partition scale column); pointwise ops run
    feature-major [32, 128] on ScalarE/VectorE.
  * Output is a single feature-major [96, 128] tensor (q rows 0:18, h_new
    32:64, c_new 64:96); the host transposes/slices.  No on-device
    transposes anywhere.
"""

import sys

if "/opt/trn_rl_repo" not in sys.path:
    sys.path.insert(0, "/opt/trn_rl_repo")

import numpy as np

B_CORE = 128          # batch per core
N_CORES = 8
CKS = [10, 18, 25, 25, 25, 25]  # batch chunk sizes (small first: pipeline fill)
CKO = [0, 10, 28, 53, 78, 103]  # chunk offsets
NCK = 6
H_BANDS = [(0, 32), (32, 32), (64, 20)]   # conv1 input-row bands (h0, nrows)

_COMPILED = None


# ---------------------------------------------------------------------------
# Host-side weight expansion
# ---------------------------------------------------------------------------

def _expand_weights(inp):
    w1 = np.asarray(inp["conv1_w"], np.float32)   # [4,4,8,8] OIHW
    b1 = np.asarray(inp["conv1_b"], np.float32)
    w2 = np.asarray(inp["conv2_w"], np.float32)   # [8,4,4,4]
    b2 = np.asarray(inp["conv2_b"], np.float32)
    w3 = np.asarray(inp["conv3_w"], np.float32)   # [8,8,3,3]
    b3 = np.asarray(inp["conv3_b"], np.float32)
    pw = np.asarray(inp["proj_w"], np.float32)    # [32, 392]
    pb = np.asarray(inp["proj_b"], np.float32)
    wih = np.asarray(inp["lstm_w_ih"], np.float32)  # [128, 51]
    whh = np.asarray(inp["lstm_w_hh"], np.float32)  # [128, 32]
    bih = np.asarray(inp["lstm_b_ih"], np.float32)
    bhh = np.asarray(inp["lstm_b_hh"], np.float32)
    qw = np.asarray(inp["q_w"], np.float32)       # [18, 32]
    qb = np.asarray(inp["q_b"], np.float32)

    # conv1 band-Toeplitz lhsT: 24 = 3 bands x 8 dx, each [128, 80]
    # partition row order is (hl, ci); M order is (i, co)
    t1 = np.zeros((128, 24 * 80), np.float32)
    for k, (h0, nr) in enumerate(H_BANDS):
        for dx in range(8):
            m = np.zeros((128, 80), np.float32)
            for ci in range(4):
                for hl in range(nr):
                    h = h0 + hl
                    for i in range(20):
                        dy = h - 4 * i
                        if 0 <= dy < 8:
                            m[hl * 4 + ci, i * 4:(i + 1) * 4] = w1[:, ci, dy, dx]
            t1[:, (k * 8 + dx) * 80:(k * 8 + dx + 1) * 80] = m

    # conv2: K=80 (=(i,co) of out1), M=72 (=(i2,co2)), 4 dx mats
    t2 = np.zeros((80, 4 * 72), np.float32)
    for dx in range(4):
        m = np.zeros((80, 72), np.float32)
        for i in range(20):
            for co in range(4):
                p = i * 4 + co
                for i2 in range(9):
                    dy = i - 2 * i2
                    if 0 <= dy < 4:
                        m[p, i2 * 8:(i2 + 1) * 8] = w2[:, co, dy, dx]
        t2[:, dx * 72:(dx + 1) * 72] = m

    # conv3: K=72 (=(i2,co2)), M=56 (=(i3,co3)), 3 dx mats
    t3 = np.zeros((72, 3 * 56), np.float32)
    for dx in range(3):
        m = np.zeros((72, 56), np.float32)
        for i2 in range(9):
            for co2 in range(8):
                p = i2 * 8 + co2
                for i3 in range(7):
                    dy = i2 - i3
                    if 0 <= dy < 3:
                        m[p, i3 * 8:(i3 + 1) * 8] = w3[:, co2, dy, dx]
        t3[:, dx * 56:(dx + 1) * 56] = m

    # proj: 7 mats [56, 32]; flatten order f = co3*49 + i3*7 + j3
    tp = np.zeros((56, 7 * 32), np.float32)
    for j3 in range(7):
        m = np.zeros((56, 32), np.float32)
        for i3 in range(7):
            for co3 in range(8):
                m[i3 * 8 + co3, :] = pw[:, co3 * 49 + i3 * 7 + j3]
        tp[:, j3 * 32:(j3 + 1) * 32] = m

    # fold prev_action / prev_reward into the gate bias
    a = int(np.asarray(inp["prev_action"]))
    a = max(0, min(a, 17))
    prev_r = float(np.asarray(inp["prev_reward"]).reshape(-1)[0])
    b_eff = bih + bhh + wih[:, 32 + a] + prev_r * wih[:, 50]

    # reorder gates (i,f,g,o) -> (i,f,o,g) so sigmoid covers rows 0:96
    perm = np.concatenate([np.arange(0, 64), np.arange(96, 128),
                           np.arange(64, 96)])
    wih = wih[perm]
    whh = whh[perm]
    b_eff = b_eff[perm]

    # per-partition bias table [128, 7]
    pbias = np.zeros((128, 7), np.float32)
    pbias[:80, 0] = np.tile(b1, 20)            # out1 bias at p=i*4+co
    pbias[:72, 1] = np.tile(b2, 9)             # out2 bias at p=i2*8+co2
    pbias[:56, 2] = np.tile(b3, 7)             # out3 bias
    pbias[:32, 3] = pb                         # proj bias
    pbias[:18, 4] = qb                         # q bias
    pbias[:, 5] = b_eff                        # gate bias (i,f,o,g rows)
    pbias[96:128, 5] *= 2.0                    # tanh-as-sigmoid: sig(2x+2b)
    pbias[:, 6] = 1.0
    pbias[96:128, 6] = 2.0                     # activation scale column

    import ml_dtypes
    bf16 = ml_dtypes.bfloat16
    wpack = np.zeros((80, 954), np.float32)
    wpack[0:80, 0:288] = t2
    wpack[0:72, 288:456] = t3
    wpack[0:56, 456:680] = tp
    wpack[0:32, 680:808] = wih[:, :32].T
    wpack[0:32, 808:936] = whh.T
    wpack[0:32, 936:954] = qw.T
    return {
        "t1": t1.astype(bf16),
        "wpack": wpack.astype(bf16),
        "pbias": pbias,
    }


# ---------------------------------------------------------------------------
# Bass kernel builder
# ---------------------------------------------------------------------------

def build_nc():
    import concourse.bass as bass
    import concourse.tile as tile
    from concourse import bacc, mybir

    f32 = mybir.dt.float32
    bf = mybir.dt.bfloat16
    AF = mybir.ActivationFunctionType

    nc = bacc.Bacc("TRN2", target_bir_lowering=False, debug=False)

    # x arrives host-pretransposed to [h, c, flat(chunk-blocked (w, bt))] bf16
    x = nc.dram_tensor("x", [84, 4, 84 * B_CORE], bf, kind="ExternalInput")
    h_in = nc.dram_tensor("h", [32, B_CORE], bf, kind="ExternalInput")
    c_in = nc.dram_tensor("c", [64, B_CORE], f32, kind="ExternalInput")
    t1 = nc.dram_tensor("t1", [128, 24 * 80], bf, kind="ExternalInput")
    wpk = nc.dram_tensor("wpack", [80, 954], bf, kind="ExternalInput")
    pbias = nc.dram_tensor("pbias", [128, 7], f32, kind="ExternalInput")
    out = nc.dram_tensor("out", [96, B_CORE], f32, kind="ExternalOutput")

    with tile.TileContext(nc) as tc:
        with tc.tile_pool(name="wpool", bufs=1) as wpool, \
             tc.tile_pool(name="xpool", bufs=3) as xpool, \
             tc.tile_pool(name="apool", bufs=2) as apool, \
             tc.tile_pool(name="spool", bufs=1) as spool, \
             tc.tile_pool(name="cps", bufs=2, space="PSUM") as cps, \
             tc.tile_pool(name="tps", bufs=1, space="PSUM") as tps:

            # ---- startup: ck0 band k interleaved with t1 slice k ----
            xb0 = []
            w1sb = wpool.tile([128, 24 * 80], bf)
            bt0 = CKS[0]
            for k, (h0, nr) in enumerate(H_BANDS):
                t = xpool.tile([4 * nr, 84 * bt0], bf, tag=f"xband{k}")
                src0 = x.ap()[h0:h0 + nr, :, 0:84 * bt0].rearrange(
                    "h c f -> (h c) f")
                nc.sync.dma_start(t[:], src0)
                nc.sync.dma_start(w1sb[:, k * 640:(k + 1) * 640],
                                  t1.ap()[:, k * 640:(k + 1) * 640])
                xb0.append(t)
            pbsb = wpool.tile([128, 7], f32)
            nc.sync.dma_start(pbsb[:], pbias.ap()[:])
            # chunk-1 bands queue ahead of the remaining (less urgent) weights
            xb1 = []
            bt1 = CKS[1]
            for k, (h0, nr) in enumerate(H_BANDS):
                t = xpool.tile([4 * nr, 84 * bt1], bf, tag=f"xband{k}")
                srck = x.ap()[h0:h0 + nr, :,
                              84 * CKO[1]:84 * (CKO[1] + bt1)].rearrange(
                    "h c f -> (h c) f")
                nc.sync.dma_start(t[:], srck)
                xb1.append(t)
            wpsb = wpool.tile([80, 954], bf)
            nc.sync.dma_start(wpsb[:], wpk.ap()[:])
            htsb = wpool.tile([32, B_CORE], bf)
            nc.sync.dma_start(htsb[:], h_in.ap()[:])
            ctsb = wpool.tile([64, B_CORE], f32)
            nc.sync.dma_start(ctsb[32:64, :], c_in.ap()[32:64, :])
            w2sb = wpsb[0:80, 0:288]
            w3sb = wpsb[0:72, 288:456]
            tpsb = wpsb[0:56, 456:680]
            wihsb = wpsb[0:32, 680:808]
            whhsb = wpsb[0:32, 808:936]
            qwsb = wpsb[0:32, 936:954]

            # conv1/conv3 output staging for the whole per-core batch
            o1all = spool.tile([80, 20, B_CORE], bf)
            o3all = spool.tile([56, 7, B_CORE], bf)

            # ---- per-chunk conv pipeline ----
            def conv1_of(ck, xb):
                bt = CKS[ck]
                bs = CKO[ck]
                z1 = cps.tile([80, 20, bt], f32, tag="z1", bufs=3)
                n_mm = 0
                for k, (h0, nr) in enumerate(H_BANDS):
                    xv = xb[k][:].rearrange("p (j s b) -> p j s b", s=4, b=bt)
                    for dx in range(8):
                        dxq, s = dx // 4, dx % 4
                        idx = k * 8 + dx
                        nc.tensor.matmul(
                            z1[:],
                            lhsT=w1sb[0:4 * nr, idx * 80:(idx + 1) * 80],
                            rhs=xv[:, dxq:dxq + 20, s, :],
                            start=(n_mm == 0), stop=(n_mm == 23))
                        n_mm += 1
                nc.scalar.activation(o1all[:, :, bs:bs + bt], z1[:], AF.Relu,
                                     bias=pbsb[0:80, 0:1])

            cfm = gps = None
            for ck in range(NCK):
                bt = CKS[ck]
                bs = CKO[ck]
                if ck == 0:
                    xb = xb0
                elif ck == 1:
                    xb = xb1
                else:
                    xb = []
                    for k, (h0, nr) in enumerate(H_BANDS):
                        t = xpool.tile([4 * nr, 84 * bt], bf, tag=f"xband{k}")
                        srck = x.ap()[h0:h0 + nr, :,
                                      84 * bs:84 * (bs + bt)].rearrange(
                            "h c f -> (h c) f")
                        nc.sync.dma_start(t[:], srck)
                        xb.append(t)

                conv1_of(ck, xb)
                if ck == 0:
                    # preload the Sigmoid LUT set off the critical tail path
                    dmy = spool.tile([1, 4], f32)
                    nc.scalar.activation(dmy[:], pbsb[0:1, 0:4], AF.Sigmoid)
                if ck == 0:
                    # the W_hh@h half of the gates, while chunk-1 x streams in
                    cfm = ctsb[32:64, :]
                    gps = tps.tile([128, B_CORE], f32, tag="g")
                    nc.tensor.matmul(gps[:], lhsT=whhsb, rhs=htsb[:],
                                     start=True, stop=False)

            # ---- conv2/conv3 over the full batch, interleaved ----
            o1v = o1all[:].rearrange("p (j s) b -> p j s b", s=2)
            o2all = spool.tile([72, 9, B_CORE], bf)

            def c2group(g0, gn):
                z2 = cps.tile([72, gn, B_CORE], f32, tag="z2")
                for dx in range(4):
                    q, s = dx // 2, dx % 2
                    nc.tensor.matmul(
                        z2[:],
                        lhsT=w2sb[:, dx * 72:(dx + 1) * 72],
                        rhs=o1v[:, g0 + q:g0 + q + gn, s, :],
                        start=(dx == 0), stop=(dx == 3))
                nc.scalar.activation(o2all[:, g0:g0 + gn, :], z2[:], AF.Relu,
                                     bias=pbsb[0:72, 1:2])

            def c3group(g0, gn):
                z3 = cps.tile([56, gn, B_CORE], f32, tag="z3")
                for dx in range(3):
                    nc.tensor.matmul(
                        z3[:],
                        lhsT=w3sb[:, dx * 56:(dx + 1) * 56],
                        rhs=o2all[:, g0 + dx:g0 + dx + gn, :],
                        start=(dx == 0), stop=(dx == 2))
                nc.scalar.activation(o3all[:, g0:g0 + gn, :], z3[:], AF.Relu,
                                     bias=pbsb[0:56, 2:3])

            c2group(0, 4)
            c2group(4, 4)
            c3group(0, 4)     # needs j2 <= 6 only: ready after groups 0-1
            c2group(8, 1)
            c3group(4, 3)

            # ---- proj: 7 matmuls over j3 -> feat [32, 128] ----
            fps = cps.tile([32, B_CORE], f32, tag="z3")
            for j3 in range(7):
                nc.tensor.matmul(
                    fps[:],
                    lhsT=tpsb[:, j3 * 32:(j3 + 1) * 32],
                    rhs=o3all[:, j3, :],
                    start=(j3 == 0), stop=(j3 == 6))
            feat = spool.tile([32, B_CORE], bf)
            nc.scalar.activation(feat[:], fps[:], AF.Identity, bias=pbsb[0:32, 3:4])

            # ---- finish LSTM gates (W_ih@feat accumulates onto W_hh@h) ----
            nc.tensor.matmul(gps[:], lhsT=wihsb, rhs=feat[:], start=False,
                             stop=True)

            # one sigmoid over all gates; tanh(g) recovered as 2*sig(2g)-1
            sgall = spool.tile([128, B_CORE], f32)
            nc.scalar.activation(sgall[:], gps[:], AF.Sigmoid,
                                 bias=pbsb[:, 5:6], scale=pbsb[:, 6:7])
            si = sgall[0:32, :]
            sf = sgall[32:64, :]
            so = sgall[64:96, :]
            tg = spool.tile([32, B_CORE], f32)
            nc.vector.tensor_scalar(tg[:], sgall[96:128, :], 2.0, -1.0,
                                    mybir.AluOpType.mult, mybir.AluOpType.add)

            ofm = spool.tile([96, B_CORE], f32)
            m1 = spool.tile([32, B_CORE], f32)
            nc.vector.tensor_mul(m1[:], si, tg[:])
            m2 = spool.tile([32, B_CORE], f32)
            nc.vector.tensor_mul(m2[:], sf, cfm)
            nc.vector.tensor_add(ofm[64:96, :], m1[:], m2[:])   # c_new
            nc.sync.dma_start(out.ap()[64:96, :], ofm[64:96, :])
            tq = spool.tile([96, B_CORE], f32)
            tcn = tq[64:96, :]
            nc.scalar.activation(tcn, ofm[64:96, :], AF.Tanh)
            nc.vector.tensor_mul(ofm[32:64, :], so, tcn)        # h_new
            nc.sync.dma_start(out.ap()[32:64, :], ofm[32:64, :])

            # ---- q head ----
            hnb = spool.tile([32, B_CORE], bf)
            nc.vector.tensor_mul(hnb[:], so, tcn)
            qps = cps.tile([18, B_CORE], f32, tag="z3")
            nc.tensor.matmul(qps[:], lhsT=qwsb, rhs=hnb[:], start=True,
                             stop=True)

            nc.scalar.activation(ofm[0:18, :], qps[:], AF.Identity,
                                 bias=pbsb[0:18, 4:5])

            nc.sync.dma_start(out.ap()[0:32, :], ofm[0:32, :])

    nc.compile()
    return nc


def _get_compiled():
    global _COMPILED
    if _COMPILED is None:
        _COMPILED = build_nc()
    return _COMPILED


# ---------------------------------------------------------------------------
# Entry point
# ---------------------------------------------------------------------------

def kernel(**inputs):
    from concourse.bass_utils import run_bass_kernel_spmd
    import ml_dtypes

    nc = _get_compiled()
    w = _expand_weights(inputs)
    bf16 = ml_dtypes.bfloat16

    x = np.asarray(inputs["x"], np.float32)
    hT = np.asarray(inputs["h"], np.float32).T          # [32, 1024]
    cT = np.asarray(inputs["c"], np.float32).T          # [32, 1024]

    in_maps = []
    for cid in range(N_CORES):
        s = slice(cid * B_CORE, (cid + 1) * B_CORE)
        # [b,c,h,w] -> [h,c,w,b], chunk-blocked flat [h,c,84*128], bf16
        x2 = x[s].transpose(2, 1, 3, 0).astype(bf16)
        xp = np.empty((84, 4, 84 * B_CORE), bf16)
        for ck in range(NCK):
            bs, bt = CKO[ck], CKS[ck]
            xp[:, :, 84 * bs:84 * (bs + bt)] = x2[:, :, :, bs:bs + bt].reshape(
                84, 4, 84 * bt)
        cpad = np.zeros((64, B_CORE), np.float32)
        cpad[32:64] = cT[:, s]
        m = {"x": xp, "h": np.ascontiguousarray(hT[:, s]).astype(bf16),
             "c": cpad}
        m.update(w)
        in_maps.append(m)

    res = run_bass_kernel_spmd(nc, in_maps, core_ids=list(range(N_CORES)))
    outs = [res.results[cid]["out"] for cid in range(N_CORES)]  # [96, 128] each
    full = np.concatenate(outs, axis=1)          # [96, 1024]
    q = np.ascontiguousarray(full[0:18].T)
    h_new = np.ascontiguousarray(full[32:64].T)
    c_new = np.ascontiguousarray(full[64:96].T)
    return q, h_new, c_new


# revision 32
# speedup vs baseline: 1.0046x; 1.0046x over previous
"""Trainium2 Bass kernel for nn_AtariAgent57LitePolicy.

Data-parallel over 8 NeuronCores (batch 1024 -> 128 per core), no collectives.

Design (per-core):
  * All convs are dense TensorE matmuls with *image rows on partitions* and
    (out-col j, batch) on the streamed free dim.  The sparse Toeplitz band
    structure of each conv is expanded on the HOST into the stationary lhsT
    operands (weights are tiny), so the device only streams activations.
  * Everything TensorE-facing is bf16 (PSUM accumulation stays f32); x is
    pre-transposed to [h, c, chunk-blocked (w, b)] and cast to bf16 on the
    host, halving the HBM floor (~7 MB/core).
  * conv1: 3 input-row bands (32/32/20 rows x 4 ci -> K=128/128/80) x 8
    kernel-columns dx -> 24 accumulating matmuls into PSUM[80=(i,co), j, b],
    per batch-chunk (sizes [10,18,25,25,25,25]; N=20*bt <= 512 per matmul).
    Band-boundary output rows receive partial sums from two bands via PSUM.
  * conv2 (k4 s2, K=80, M=72) and conv3 (k3 s1, K=72, M=56) run once over
    the full 128 batch at the end (12 + 6 matmuls), then proj (7 matmuls)
    -> conv_feat [32, 128].
  * LSTM: h/c arrive host-transposed (feature-major); W_hh@h issues during
    chunk 0; prev_action one-hot, prev_reward, and both LSTM biases fold
    into one gate-bias vector on the host.  Gates are host-permuted to
    (i,f,o,g) and evaluated with a single Sigmoid pass (tanh(g) recovered
    as 2*sigmoid(2g)-1 via a per-partition scale column); pointwise ops run
    feature-major [32, 128] on ScalarE/VectorE.
  * Output is a single feature-major [96, 128] tensor (q rows 0:18, h_new
    32:64, c_new 64:96); the host transposes/slices.  No on-device
    transposes anywhere.
"""

import sys

if "/opt/trn_rl_repo" not in sys.path:
    sys.path.insert(0, "/opt/trn_rl_repo")

import numpy as np

B_CORE = 128          # batch per core
N_CORES = 8
CKS = [10, 18, 25, 25, 25, 25]  # batch chunk sizes (small first: pipeline fill)
CKO = [0, 10, 28, 53, 78, 103]  # chunk offsets
NCK = 6
H_BANDS = [(0, 32), (32, 32), (64, 20)]   # conv1 input-row bands (h0, nrows)

_COMPILED = None


# ---------------------------------------------------------------------------
# Host-side weight expansion
# ---------------------------------------------------------------------------

def _expand_weights(inp):
    w1 = np.asarray(inp["conv1_w"], np.float32)   # [4,4,8,8] OIHW
    b1 = np.asarray(inp["conv1_b"], np.float32)
    w2 = np.asarray(inp["conv2_w"], np.float32)   # [8,4,4,4]
    b2 = np.asarray(inp["conv2_b"], np.float32)
    w3 = np.asarray(inp["conv3_w"], np.float32)   # [8,8,3,3]
    b3 = np.asarray(inp["conv3_b"], np.float32)
    pw = np.asarray(inp["proj_w"], np.float32)    # [32, 392]
    pb = np.asarray(inp["proj_b"], np.float32)
    wih = np.asarray(inp["lstm_w_ih"], np.float32)  # [128, 51]
    whh = np.asarray(inp["lstm_w_hh"], np.float32)  # [128, 32]
    bih = np.asarray(inp["lstm_b_ih"], np.float32)
    bhh = np.asarray(inp["lstm_b_hh"], np.float32)
    qw = np.asarray(inp["q_w"], np.float32)       # [18, 32]
    qb = np.asarray(inp["q_b"], np.float32)

    # conv1 band-Toeplitz lhsT: 24 = 3 bands x 8 dx, each [128, 80]
    # partition row order is (hl, ci); M order is (i, co)
    t1 = np.zeros((128, 24 * 80), np.float32)
    for k, (h0, nr) in enumerate(H_BANDS):
        for dx in range(8):
            m = np.zeros((128, 80), np.float32)
            for ci in range(4):
                for hl in range(nr):
                    h = h0 + hl
                    for i in range(20):
                        dy = h - 4 * i
                        if 0 <= dy < 8:
                            m[hl * 4 + ci, i * 4:(i + 1) * 4] = w1[:, ci, dy, dx]
            t1[:, (k * 8 + dx) * 80:(k * 8 + dx + 1) * 80] = m

    # conv2: K=80 (=(i,co) of out1), M=72 (=(i2,co2)), 4 dx mats
    t2 = np.zeros((80, 4 * 72), np.float32)
    for dx in range(4):
        m = np.zeros((80, 72), np.float32)
        for i in range(20):
            for co in range(4):
                p = i * 4 + co
                for i2 in range(9):
                    dy = i - 2 * i2
                    if 0 <= dy < 4:
                        m[p, i2 * 8:(i2 + 1) * 8] = w2[:, co, dy, dx]
        t2[:, dx * 72:(dx + 1) * 72] = m

    # conv3: K=72 (=(i2,co2)), M=56 (=(i3,co3)), 3 dx mats
    t3 = np.zeros((72, 3 * 56), np.float32)
    for dx in range(3):
        m = np.zeros((72, 56), np.float32)
        for i2 in range(9):
            for co2 in range(8):
                p = i2 * 8 + co2
                for i3 in range(7):
                    dy = i2 - i3
                    if 0 <= dy < 3:
                        m[p, i3 * 8:(i3 + 1) * 8] = w3[:, co2, dy, dx]
        t3[:, dx * 56:(dx + 1) * 56] = m

    # proj: 7 mats [56, 32]; flatten order f = co3*49 + i3*7 + j3
    tp = np.zeros((56, 7 * 32), np.float32)
    for j3 in range(7):
        m = np.zeros((56, 32), np.float32)
        for i3 in range(7):
            for co3 in range(8):
                m[i3 * 8 + co3, :] = pw[:, co3 * 49 + i3 * 7 + j3]
        tp[:, j3 * 32:(j3 + 1) * 32] = m

    # fold prev_action / prev_reward into the gate bias
    a = int(np.asarray(inp["prev_action"]))
    a = max(0, min(a, 17))
    prev_r = float(np.asarray(inp["prev_reward"]).reshape(-1)[0])
    b_eff = bih + bhh + wih[:, 32 + a] + prev_r * wih[:, 50]

    # reorder gates (i,f,g,o) -> (i,f,o,g) so sigmoid covers rows 0:96
    perm = np.concatenate([np.arange(0, 64), np.arange(96, 128),
                           np.arange(64, 96)])
    wih = wih[perm]
    whh = whh[perm]
    b_eff = b_eff[perm]

    # per-partition bias table [128, 7]
    pbias = np.zeros((128, 7), np.float32)
    pbias[:80, 0] = np.tile(b1, 20)            # out1 bias at p=i*4+co
    pbias[:72, 1] = np.tile(b2, 9)             # out2 bias at p=i2*8+co2
    pbias[:56, 2] = np.tile(b3, 7)             # out3 bias
    pbias[:32, 3] = pb                         # proj bias
    pbias[:18, 4] = qb                         # q bias
    pbias[:, 5] = b_eff                        # gate bias (i,f,o,g rows)
    pbias[96:128, 5] *= 2.0                    # tanh-as-sigmoid: sig(2x+2b)
    pbias[:, 6] = 1.0
    pbias[96:128, 6] = 2.0                     # activation scale column

    import ml_dtypes
    bf16 = ml_dtypes.bfloat16
    wpack = np.zeros((80, 954), np.float32)
    wpack[0:80, 0:288] = t2
    wpack[0:72, 288:456] = t3
    wpack[0:56, 456:680] = tp
    wpack[0:32, 680:808] = wih[:, :32].T
    wpack[0:32, 808:936] = whh.T
    wpack[0:32, 936:954] = qw.T
    return {
        "t1": t1.astype(bf16),
        "wpack": wpack.astype(bf16),
        "pbias": pbias,
    }


# ---------------------------------------------------------------------------
# Bass kernel builder
# ---------------------------------------------------------------------------

def build_nc():
    import concourse.bass as bass
    import concourse.tile as tile
    from concourse import bacc, mybir

    f32 = mybir.dt.float32
    bf = mybir.dt.bfloat16
    AF = mybir.ActivationFunctionType

    nc = bacc.Bacc("TRN2", target_bir_lowering=False, debug=False)

    # x arrives host-pretransposed to [h, c, flat(chunk-blocked (w, bt))] bf16
    x = nc.dram_tensor("x", [84, 4, 84 * B_CORE], bf, kind="ExternalInput")
    h_in = nc.dram_tensor("h", [32, B_CORE], bf, kind="ExternalInput")
    c_in = nc.dram_tensor("c", [64, B_CORE], f32, kind="ExternalInput")
    t1 = nc.dram_tensor("t1", [128, 24 * 80], bf, kind="ExternalInput")
    wpk = nc.dram_tensor("wpack", [80, 954], bf, kind="ExternalInput")
    pbias = nc.dram_tensor("pbias", [128, 7], f32, kind="ExternalInput")
    out = nc.dram_tensor("out", [96, B_CORE], f32, kind="ExternalOutput")

    with tile.TileContext(nc) as tc:
        with tc.tile_pool(name="wpool", bufs=1) as wpool, \
             tc.tile_pool(name="xpool", bufs=3) as xpool, \
             tc.tile_pool(name="apool", bufs=2) as apool, \
             tc.tile_pool(name="spool", bufs=1) as spool, \
             tc.tile_pool(name="cps", bufs=2, space="PSUM") as cps, \
             tc.tile_pool(name="tps", bufs=1, space="PSUM") as tps:

            # ---- startup: ck0 band k interleaved with t1 slice k ----
            xb0 = []
            w1sb = wpool.tile([128, 24 * 80], bf)
            bt0 = CKS[0]
            for k, (h0, nr) in enumerate(H_BANDS):
                t = xpool.tile([4 * nr, 84 * bt0], bf, tag=f"xband{k}")
                src0 = x.ap()[h0:h0 + nr, :, 0:84 * bt0].rearrange(
                    "h c f -> (h c) f")
                nc.sync.dma_start(t[:], src0)
                nc.sync.dma_start(w1sb[:, k * 640:(k + 1) * 640],
                                  t1.ap()[:, k * 640:(k + 1) * 640])
                xb0.append(t)
            pbsb = wpool.tile([128, 7], f32)
            nc.sync.dma_start(pbsb[:], pbias.ap()[:])
            # chunk-1 bands queue ahead of the remaining (less urgent) weights
            xb1 = []
            bt1 = CKS[1]
            for k, (h0, nr) in enumerate(H_BANDS):
                t = xpool.tile([4 * nr, 84 * bt1], bf, tag=f"xband{k}")
                srck = x.ap()[h0:h0 + nr, :,
                              84 * CKO[1]:84 * (CKO[1] + bt1)].rearrange(
                    "h c f -> (h c) f")
                nc.sync.dma_start(t[:], srck)
                xb1.append(t)
            wpsb = wpool.tile([80, 954], bf)
            nc.sync.dma_start(wpsb[:], wpk.ap()[:])
            htsb = wpool.tile([32, B_CORE], bf)
            nc.sync.dma_start(htsb[:], h_in.ap()[:])
            ctsb = wpool.tile([64, B_CORE], f32)
            nc.sync.dma_start(ctsb[32:64, :], c_in.ap()[32:64, :])
            w2sb = wpsb[0:80, 0:288]
            w3sb = wpsb[0:72, 288:456]
            tpsb = wpsb[0:56, 456:680]
            wihsb = wpsb[0:32, 680:808]
            whhsb = wpsb[0:32, 808:936]
            qwsb = wpsb[0:32, 936:954]

            # conv1/conv3 output staging for the whole per-core batch
            o1all = spool.tile([80, 20, B_CORE], bf)
            o3all = spool.tile([56, 7, B_CORE], bf)

            # ---- per-chunk conv pipeline ----
            def conv1_of(ck, xb):
                bt = CKS[ck]
                bs = CKO[ck]
                z1 = cps.tile([80, 20, bt], f32, tag="z1", bufs=3)
                n_mm = 0
                for k, (h0, nr) in enumerate(H_BANDS):
                    xv = xb[k][:].rearrange("p (j s b) -> p j s b", s=4, b=bt)
                    for dx in range(8):
                        dxq, s = dx // 4, dx % 4
                        idx = k * 8 + dx
                        nc.tensor.matmul(
                            z1[:],
                            lhsT=w1sb[0:4 * nr, idx * 80:(idx + 1) * 80],
                            rhs=xv[:, dxq:dxq + 20, s, :],
                            start=(n_mm == 0), stop=(n_mm == 23))
                        n_mm += 1
                if ck == NCK - 1:
                    # split the last relu by j-halves: conv2 group 0 only
                    # needs j <= 9, so it can start after the first half
                    nc.scalar.activation(o1all[:, 0:10, bs:bs + bt],
                                         z1[:, 0:10, :], AF.Relu,
                                         bias=pbsb[0:80, 0:1])
                    nc.scalar.activation(o1all[:, 10:20, bs:bs + bt],
                                         z1[:, 10:20, :], AF.Relu,
                                         bias=pbsb[0:80, 0:1])
                else:
                    nc.scalar.activation(o1all[:, :, bs:bs + bt], z1[:],
                                         AF.Relu, bias=pbsb[0:80, 0:1])

            cfm = gps = None
            for ck in range(NCK):
                bt = CKS[ck]
                bs = CKO[ck]
                if ck == 0:
                    xb = xb0
                elif ck == 1:
                    xb = xb1
                else:
                    xb = []
                    for k, (h0, nr) in enumerate(H_BANDS):
                        t = xpool.tile([4 * nr, 84 * bt], bf, tag=f"xband{k}")
                        srck = x.ap()[h0:h0 + nr, :,
                                      84 * bs:84 * (bs + bt)].rearrange(
                            "h c f -> (h c) f")
                        nc.sync.dma_start(t[:], srck)
                        xb.append(t)

                conv1_of(ck, xb)
                if ck == 0:
                    # preload the Sigmoid LUT set off the critical tail path
                    dmy = spool.tile([1, 4], f32)
                    nc.scalar.activation(dmy[:], pbsb[0:1, 0:4], AF.Sigmoid)
                if ck == 0:
                    # the W_hh@h half of the gates, while chunk-1 x streams in
                    cfm = ctsb[32:64, :]
                    gps = tps.tile([128, B_CORE], f32, tag="g")
                    nc.tensor.matmul(gps[:], lhsT=whhsb, rhs=htsb[:],
                                     start=True, stop=False)

            # ---- conv2/conv3 over the full batch, interleaved ----
            o1v = o1all[:].rearrange("p (j s) b -> p j s b", s=2)
            o2all = spool.tile([72, 9, B_CORE], bf)

            def c2group(g0, gn):
                z2 = cps.tile([72, gn, B_CORE], f32, tag="z2")
                for dx in range(4):
                    q, s = dx // 2, dx % 2
                    nc.tensor.matmul(
                        z2[:],
                        lhsT=w2sb[:, dx * 72:(dx + 1) * 72],
                        rhs=o1v[:, g0 + q:g0 + q + gn, s, :],
                        start=(dx == 0), stop=(dx == 3))
                nc.scalar.activation(o2all[:, g0:g0 + gn, :], z2[:], AF.Relu,
                                     bias=pbsb[0:72, 1:2])

            def c3group(g0, gn):
                z3 = cps.tile([56, gn, B_CORE], f32, tag="z3")
                for dx in range(3):
                    nc.tensor.matmul(
                        z3[:],
                        lhsT=w3sb[:, dx * 56:(dx + 1) * 56],
                        rhs=o2all[:, g0 + dx:g0 + dx + gn, :],
                        start=(dx == 0), stop=(dx == 2))
                nc.scalar.activation(o3all[:, g0:g0 + gn, :], z3[:], AF.Relu,
                                     bias=pbsb[0:56, 2:3])

            c2group(0, 4)
            c2group(4, 4)
            c3group(0, 4)     # needs j2 <= 6 only: ready after groups 0-1
            c2group(8, 1)
            c3group(4, 3)

            # ---- proj: 7 matmuls over j3 -> feat [32, 128] ----
            fps = cps.tile([32, B_CORE], f32, tag="z3")
            for j3 in range(7):
                nc.tensor.matmul(
                    fps[:],
                    lhsT=tpsb[:, j3 * 32:(j3 + 1) * 32],
                    rhs=o3all[:, j3, :],
                    start=(j3 == 0), stop=(j3 == 6))
            feat = spool.tile([32, B_CORE], bf)
            nc.scalar.activation(feat[:], fps[:], AF.Identity, bias=pbsb[0:32, 3:4])

            # ---- finish LSTM gates (W_ih@feat accumulates onto W_hh@h) ----
            nc.tensor.matmul(gps[:], lhsT=wihsb, rhs=feat[:], start=False,
                             stop=True)

            # one sigmoid over all gates; tanh(g) recovered as 2*sig(2g)-1
            sgall = spool.tile([128, B_CORE], f32)
            nc.scalar.activation(sgall[:], gps[:], AF.Sigmoid,
                                 bias=pbsb[:, 5:6], scale=pbsb[:, 6:7])
            si = sgall[0:32, :]
            sf = sgall[32:64, :]
            so = sgall[64:96, :]
            tg = spool.tile([32, B_CORE], f32)
            nc.vector.tensor_scalar(tg[:], sgall[96:128, :], 2.0, -1.0,
                                    mybir.AluOpType.mult, mybir.AluOpType.add)

            ofm = spool.tile([96, B_CORE], f32)
            m1 = spool.tile([32, B_CORE], f32)
            nc.vector.tensor_mul(m1[:], si, tg[:])
            m2 = spool.tile([32, B_CORE], f32)
            nc.vector.tensor_mul(m2[:], sf, cfm)
            nc.vector.tensor_add(ofm[64:96, :], m1[:], m2[:])   # c_new
            nc.sync.dma_start(out.ap()[64:96, :], ofm[64:96, :])
            tq = spool.tile([96, B_CORE], f32)
            tcn = tq[64:96, :]
            nc.scalar.activation(tcn, ofm[64:96, :], AF.Tanh)
            nc.vector.tensor_mul(ofm[32:64, :], so, tcn)        # h_new
            nc.sync.dma_start(out.ap()[32:64, :], ofm[32:64, :])

            # ---- q head ----
            hnb = spool.tile([32, B_CORE], bf)
            nc.vector.tensor_mul(hnb[:], so, tcn)
            qps = cps.tile([18, B_CORE], f32, tag="z3")
            nc.tensor.matmul(qps[:], lhsT=qwsb, rhs=hnb[:], start=True,
                             stop=True)

            nc.scalar.activation(ofm[0:18, :], qps[:], AF.Identity,
                                 bias=pbsb[0:18, 4:5])

            nc.sync.dma_start(out.ap()[0:32, :], ofm[0:32, :])

    nc.compile()
    return nc


def _get_compiled():
    global _COMPILED
    if _COMPILED is None:
        _COMPILED = build_nc()
    return _COMPILED


# ---------------------------------------------------------------------------
# Entry point
# ---------------------------------------------------------------------------

def kernel(**inputs):
    from concourse.bass_utils import run_bass_kernel_spmd
    import ml_dtypes

    nc = _get_compiled()
    w = _expand_weights(inputs)
    bf16 = ml_dtypes.bfloat16

    x = np.asarray(inputs["x"], np.float32)
    hT = np.asarray(inputs["h"], np.float32).T          # [32, 1024]
    cT = np.asarray(inputs["c"], np.float32).T          # [32, 1024]

    in_maps = []
    for cid in range(N_CORES):
        s = slice(cid * B_CORE, (cid + 1) * B_CORE)
        # [b,c,h,w] -> [h,c,w,b], chunk-blocked flat [h,c,84*128], bf16
        x2 = x[s].transpose(2, 1, 3, 0).astype(bf16)
        xp = np.empty((84, 4, 84 * B_CORE), bf16)
        for ck in range(NCK):
            bs, bt = CKO[ck], CKS[ck]
            xp[:, :, 84 * bs:84 * (bs + bt)] = x2[:, :, :, bs:bs + bt].reshape(
                84, 4, 84 * bt)
        cpad = np.zeros((64, B_CORE), np.float32)
        cpad[32:64] = cT[:, s]
        m = {"x": xp, "h": np.ascontiguousarray(hT[:, s]).astype(bf16),
             "c": cpad}
        m.update(w)
        in_maps.append(m)

    res = run_bass_kernel_spmd(nc, in_maps, core_ids=list(range(N_CORES)))
    outs = [res.results[cid]["out"] for cid in range(N_CORES)]  # [96, 128] each
    full = np.concatenate(outs, axis=1)          # [96, 1024]
    q = np.ascontiguousarray(full[0:18].T)
    h_new = np.ascontiguousarray(full[32:64].T)
    c_new = np.ascontiguousarray(full[64:96].T)
    return q, h_new, c_new
